# revision 18
# baseline (speedup 1.0000x reference)
"""Trainium2 Bass kernel: 3x3 conv (NCHW 32x256x56x56, 256->256ch, pad 1) with
a host-expanded synthesized weight, data-parallel over 8 NeuronCores.

Conv as implicit GEMM: for each of the 9 kernel taps, a matmul over a
zero-padded (58x58, padded on host) input image held in SBUF with input
channels on partitions, accumulating over 18 matmuls (9 taps x 2 channel
tiles) in PSUM.  fp16 operands (fp32 accumulate) keep the PE at 1 col/cycle
with LDWEIGHTS fully hidden via fast-weight-load; N = 8 rows x 56 cols = 448
per matmul (PSUM-bank limit is 512 fp32).  ~75 junk warmup matmuls flip the
HAM clock gate to 8/8 during the head DMA wait.  Head DMAs are ordered so
the first 18 real matmuls never stall (weights land whole per (kt,mt) ahead
of first use; stalls re-throttle the PE clock and cost double).  Output is
converted to fp16 on the DVE during the bias add (host converts back to
fp32), halving output DMA bytes.  Measured floor: 1008 matmuls x 448 cols
@2.4GHz = ~190us of PE streaming.
"""

import numpy as np

# Problem constants (hardcoded per contract; kernel.py must be self-contained)
OOC, OIC, K1, K2 = 64, 64, 3, 3
R0, R1 = 4, 4
N_CORES = 8
BATCH = 32
N_PER_CORE = BATCH // N_CORES  # 4
C = 256
H = W = 56
HP = WP = H + 2  # zero-padded spatial (padding applied on host)
RB = 8           # output rows per matmul chunk -> N = RB*W = 448
NCH = H // RB    # 7 chunks
KT = C // 128    # 2 input-channel tiles
MT = C // 128    # 2 output-channel tiles
POS = K1 * K2    # 9 kernel taps

_NC_CACHE = {}
LAST_RESULT = {}  # test.py introspection: last BassKernelResults


def _expand_weight(weight, alphas, betas):
    """W[p0*64+i, p1*64+j, ky, kx] = w[i,j,ky,kx] * a[p0,p1] / (1+exp(w*b[p0,p1]))."""
    w = weight.astype(np.float32)[None, None]            # (1,1,64,64,3,3)
    a = alphas.astype(np.float32).reshape(R0, R1)[:, :, None, None, None, None]
    b = betas.astype(np.float32).reshape(R0, R1)[:, :, None, None, None, None]
    act = w * a / (1.0 + np.exp(w * b))                  # (4,4,64,64,3,3)
    return act.transpose(0, 2, 1, 3, 4, 5).reshape(R0 * OOC, R1 * OIC, K1, K2)


def _host_prep(x, weight, alphas, betas, bias):
    x = np.asarray(x, dtype=np.float32).astype(np.float16)
    xpad = np.pad(x, ((0, 0), (0, 0), (1, 1), (1, 1)))
    Wfull = _expand_weight(np.asarray(weight), np.asarray(alphas),
                           np.asarray(betas))            # (256,256,3,3)
    # lhsT layout: [ci_local(128 partitions), kt, mt, pos, co_local(128)]
    Wt = Wfull.transpose(1, 0, 2, 3).reshape(C, C, POS)  # (ci, co, pos)
    w_arr = np.ascontiguousarray(
        Wt.reshape(KT, 128, MT, 128, POS).transpose(1, 0, 2, 4, 3)
    ).astype(np.float16)
    b_arr = np.ascontiguousarray(
        np.asarray(bias, dtype=np.float32).reshape(MT, 128).T)
    return xpad, w_arr, b_arr


def _build_nc():
    import concourse.mybir as mybir
    import concourse.tile as tile
    from concourse import bacc

    fp32 = mybir.dt.float32
    fp16 = mybir.dt.float16

    nc = bacc.Bacc("TRN2", target_bir_lowering=False, debug=False,
                   num_devices=N_CORES)

    x_d = nc.dram_tensor("x", [N_PER_CORE, C, HP, WP], fp16,
                         kind="ExternalInput")
    w_d = nc.dram_tensor("w", [128, KT, MT, POS, 128], fp16,
                         kind="ExternalInput")
    b_d = nc.dram_tensor("b", [128, MT], fp32, kind="ExternalInput")
    o_d = nc.dram_tensor("out", [N_PER_CORE, C, H, W], fp16,
                         kind="ExternalOutput")

    # Two HWDGE rings: sync carries kt=0 input traffic + mt=0 outputs,
    # scalar carries kt=1 inputs + mt=1 outputs.
    def ring(kt):
        return nc.sync if kt == 0 else nc.scalar

    with tile.TileContext(nc) as tc:
        with (
            tc.tile_pool(name="const", bufs=1) as const_pool,
            tc.tile_pool(name="xpad", bufs=1) as xp_pool,
            tc.tile_pool(name="ot", bufs=4) as out_pool,
            tc.tile_pool(name="ps", bufs=7, space="PSUM") as psum_pool,
        ):
            w_sb = const_pool.tile([128, KT, MT, POS, 128], fp16,
                                   name="w_sb", tag="w_sb")
            b_sb = const_pool.tile([128, MT], fp32, name="b_sb", tag="b_sb")

            # PE warmup: ~4us of junk matmuls on scratch SBUF during the
            # initial DMA wait flips the HAM clock gate to 8/8 before the
            # real stream starts (and costs nothing - PE is idle anyway).
            warm_in = const_pool.tile([128, 128], fp16, name="warm_in",
                                      tag="warm_in")
            warm_ps = psum_pool.tile([128, 64], fp32, name="warm_ps",
                                     tag="warm_ps", bufs=1)
            nc.gpsimd.memset(warm_in[:], 0.0)
            for _ in range(62):
                nc.tensor.matmul(warm_ps[:], warm_in[:], warm_in[:, 0:64])

            # Double-buffered padded input images (pad arrives from host).
            xp = [[xp_pool.tile([128, HP, WP], fp16, name=f"xp{par}_{kt}",
                                tag=f"xp{par}_{kt}")
                   for kt in range(KT)] for par in range(2)]

            xap = x_d.ap()
            oap = o_d.ap()
            wap = w_d.ap()

            def xdma(eng, n, par, kt, r0, r1):
                eng.dma_start(xp[par][kt][:, r0:r1, :],
                              xap[n, kt * 128:(kt + 1) * 128, r0:r1, :])

            # Head: completions on a ring serialize at ~1.5-2us each, and a
            # mid-stream matmul stall also re-throttles the PE clock for
            # ~3.4us at half rate.  The four operands of the first 18 real
            # matmuls (band-0 of both kt, mt0 weights of both kt) are spread
            # over FOUR queues (sync/scalar HWDGE + gpsimd/vector SWDGE) so
            # they complete in parallel ~2us before first use.
            xdma(nc.sync, 0, 0, 0, 0, 18)                      # band0 kt0
            nc.scalar.dma_start(w_sb[:, 0, 0], wap[:, 0, 0])   # w kt0 mt0
            xdma(nc.gpsimd, 0, 0, 1, 0, 18)                    # band0 kt1
            nc.scalar.dma_start(w_sb[:, 1, 0], wap[:, 1, 0])   # w kt1 mt0
            nc.sync.dma_start(w_sb[:, 0, 1], wap[:, 0, 1])     # w kt0 mt1
            nc.scalar.dma_start(w_sb[:, 1, 1], wap[:, 1, 1])   # w kt1 mt1
            xdma(nc.sync, 0, 0, 0, 18, 34)                     # rows 18:34
            xdma(nc.scalar, 0, 0, 1, 18, 34)
            nc.sync.dma_start(b_sb[:], b_d.ap())               # bias
            xdma(nc.scalar, 0, 0, 1, 34, 58)                   # rows 34:58
            xdma(nc.sync, 0, 0, 0, 34, 58)

            def drain(n, mt, y0, ps, rows=RB):
                ot = out_pool.tile([128, rows, W], fp16, name="ot", tag="ot")
                nc.vector.tensor_scalar_add(ot[:], ps[:], b_sb[:, mt:mt + 1])
                ring(mt).dma_start(
                    oap[n, mt * 128:(mt + 1) * 128, y0:y0 + rows, :], ot[:])

            def chunk_mms(par, mt, y0, ps, rows=RB):
                first = True
                for kt in range(KT):
                    for dy in range(K1):
                        for dx in range(K2):
                            pos = dy * K2 + dx
                            last = (kt == KT - 1 and pos == POS - 1)
                            nc.tensor.matmul(
                                ps[:, :, :],
                                w_sb[:, kt, mt, pos, :],
                                xp[par][kt][:, y0 + dy:y0 + dy + rows,
                                            dx:dx + W],
                                start=first, stop=last,
                            )
                            first = False

            # Chunks 0+1 of image 0 open the stream: band-0 (rows 0:18)
            # covers both, so their eight (ch, mt, kt) blocks interleave
            # kt-major across four PSUM banks — each later head operand
            # (w mt1, band kt1, w kt1) is needed another ~2-3.5us of real
            # work later, absorbing per-queue DMA completion jitter without
            # junk-matmul padding.
            ps_c0 = [[psum_pool.tile([128, RB, W], fp32, name="ps", tag="ps")
                      for _ in range(MT)] for _ in range(2)]
            for kt in range(KT):
                for mt in range(MT):
                    for c in range(2):
                        for dy in range(K1):
                            for dx in range(K2):
                                pos = dy * K2 + dx
                                nc.tensor.matmul(
                                    ps_c0[c][mt][:, :, :],
                                    w_sb[:, kt, mt, pos, :],
                                    xp[0][kt][:, c * RB + dy:c * RB + dy + RB,
                                              dx:dx + W],
                                    start=(kt == 0 and pos == 0),
                                    stop=(kt == KT - 1 and pos == POS - 1),
                                )
                    # Small junk cushion: insurance against the next block's
                    # operand DMA still being in flight (a stalled PE idles
                    # AND can re-throttle the clock gate).
                    for _ in range(4):
                        nc.tensor.matmul(warm_ps[:], warm_in[:],
                                         warm_in[:, 0:64])
            for c in range(2):
                for mt in range(MT):
                    drain(0, mt, c * RB, ps_c0[c][mt])

            for n in range(N_PER_CORE):
                par = n % 2
                if n > 0:
                    for kt in range(KT):
                        xdma(ring(kt), n, par, kt, 0, 58)      # whole image
                for ch in range(2 if n == 0 else 0, NCH):
                    y0 = ch * RB
                    # Final chunk of the final image: split into half-chunks
                    # so the last drain + output DMA (on the span's critical
                    # path: ~0.7us vector + ~2.6us DMA round-trip) overlaps
                    # the trailing matmuls.
                    halve = (n == N_PER_CORE - 1 and ch == NCH - 1)
                    for mt in range(MT):
                        if halve:
                            for h in range(2):
                                hr = RB // 2
                                ps = psum_pool.tile([128, hr, W], fp32,
                                                    name="ps", tag="ps")
                                chunk_mms(par, mt, y0 + h * hr, ps, rows=hr)
                                drain(n, mt, y0 + h * hr, ps, rows=hr)
                        else:
                            ps = psum_pool.tile([128, RB, W], fp32,
                                                name="ps", tag="ps")
                            chunk_mms(par, mt, y0, ps)
                            drain(n, mt, y0, ps)
    nc.compile()
    return nc


def get_nc():
    if "nc" not in _NC_CACHE:
        _NC_CACHE["nc"] = _build_nc()
    return _NC_CACHE["nc"]


def kernel(x, weight, alphas, betas, bias):
    from concourse.bass_utils import run_bass_kernel_spmd

    xpad, w_arr, b_arr = _host_prep(x, weight, alphas, betas, bias)
    nc = get_nc()
    in_maps = [
        {"x": xpad[i * N_PER_CORE:(i + 1) * N_PER_CORE], "w": w_arr,
         "b": b_arr}
        for i in range(N_CORES)
    ]
    res = run_bass_kernel_spmd(nc, in_maps, core_ids=list(range(N_CORES)))
    LAST_RESULT["res"] = res
    return np.concatenate([r["out"] for r in res.results],
                          axis=0).astype(np.float32)


# revision 20
# speedup vs baseline: 1.0132x; 1.0132x over previous
"""Trainium2 Bass kernel: 3x3 conv (NCHW 32x256x56x56, 256->256ch, pad 1) with
a host-expanded synthesized weight, data-parallel over 8 NeuronCores.

Conv as implicit GEMM: for each of the 9 kernel taps, a matmul over a
zero-padded (58x58, padded on host) input image held in SBUF with input
channels on partitions, accumulating over 18 matmuls (9 taps x 2 channel
tiles) in PSUM.  fp16 operands (fp32 accumulate) keep the PE at 1 col/cycle
with LDWEIGHTS fully hidden via fast-weight-load; N = 8 rows x 56 cols = 448
per matmul (PSUM-bank limit is 512 fp32).  ~75 junk warmup matmuls flip the
HAM clock gate to 8/8 during the head DMA wait.  Head DMAs are ordered so
the first 18 real matmuls never stall (weights land whole per (kt,mt) ahead
of first use; stalls re-throttle the PE clock and cost double).  Output is
converted to fp16 on the DVE during the bias add (host converts back to
fp32), halving output DMA bytes.  Measured floor: 1008 matmuls x 448 cols
@2.4GHz = ~190us of PE streaming.
"""

import numpy as np

# Problem constants (hardcoded per contract; kernel.py must be self-contained)
OOC, OIC, K1, K2 = 64, 64, 3, 3
R0, R1 = 4, 4
N_CORES = 8
BATCH = 32
N_PER_CORE = BATCH // N_CORES  # 4
C = 256
H = W = 56
HP = WP = H + 2  # zero-padded spatial (padding applied on host)
RB = 8           # output rows per matmul chunk -> N = RB*W = 448
NCH = H // RB    # 7 chunks
KT = C // 128    # 2 input-channel tiles
MT = C // 128    # 2 output-channel tiles
POS = K1 * K2    # 9 kernel taps

_NC_CACHE = {}
LAST_RESULT = {}  # test.py introspection: last BassKernelResults


def _expand_weight(weight, alphas, betas):
    """W[p0*64+i, p1*64+j, ky, kx] = w[i,j,ky,kx] * a[p0,p1] / (1+exp(w*b[p0,p1]))."""
    w = weight.astype(np.float32)[None, None]            # (1,1,64,64,3,3)
    a = alphas.astype(np.float32).reshape(R0, R1)[:, :, None, None, None, None]
    b = betas.astype(np.float32).reshape(R0, R1)[:, :, None, None, None, None]
    act = w * a / (1.0 + np.exp(w * b))                  # (4,4,64,64,3,3)
    return act.transpose(0, 2, 1, 3, 4, 5).reshape(R0 * OOC, R1 * OIC, K1, K2)


def _host_prep(x, weight, alphas, betas, bias):
    x = np.asarray(x, dtype=np.float32).astype(np.float16)
    xpad = np.pad(x, ((0, 0), (0, 0), (1, 1), (1, 1)))
    Wfull = _expand_weight(np.asarray(weight), np.asarray(alphas),
                           np.asarray(betas))            # (256,256,3,3)
    # lhsT layout: [ci_local(128 partitions), kt, mt, pos, co_local(128)]
    Wt = Wfull.transpose(1, 0, 2, 3).reshape(C, C, POS)  # (ci, co, pos)
    w_arr = np.ascontiguousarray(
        Wt.reshape(KT, 128, MT, 128, POS).transpose(1, 0, 2, 4, 3)
    ).astype(np.float16)
    b_arr = np.ascontiguousarray(
        np.asarray(bias, dtype=np.float32).reshape(MT, 128).T)
    return xpad, w_arr, b_arr


def _build_nc():
    import concourse.mybir as mybir
    import concourse.tile as tile
    from concourse import bacc

    fp32 = mybir.dt.float32
    fp16 = mybir.dt.float16

    nc = bacc.Bacc("TRN2", target_bir_lowering=False, debug=False,
                   num_devices=N_CORES)

    x_d = nc.dram_tensor("x", [N_PER_CORE, C, HP, WP], fp16,
                         kind="ExternalInput")
    w_d = nc.dram_tensor("w", [128, KT, MT, POS, 128], fp16,
                         kind="ExternalInput")
    b_d = nc.dram_tensor("b", [128, MT], fp32, kind="ExternalInput")
    o_d = nc.dram_tensor("out", [N_PER_CORE, C, H, W], fp16,
                         kind="ExternalOutput")

    # Two HWDGE rings: sync carries kt=0 input traffic + mt=0 outputs,
    # scalar carries kt=1 inputs + mt=1 outputs.
    def ring(kt):
        return nc.sync if kt == 0 else nc.scalar

    with tile.TileContext(nc) as tc:
        with (
            tc.tile_pool(name="const", bufs=1) as const_pool,
            tc.tile_pool(name="xpad", bufs=1) as xp_pool,
            tc.tile_pool(name="ot", bufs=4) as out_pool,
            tc.tile_pool(name="ps", bufs=7, space="PSUM") as psum_pool,
        ):
            w_sb = const_pool.tile([128, KT, MT, POS, 128], fp16,
                                   name="w_sb", tag="w_sb")
            b_sb = const_pool.tile([128, MT], fp32, name="b_sb", tag="b_sb")

            # PE warmup: ~4us of junk matmuls on scratch SBUF during the
            # initial DMA wait flips the HAM clock gate to 8/8 before the
            # real stream starts (and costs nothing - PE is idle anyway).
            warm_in = const_pool.tile([128, 128], fp16, name="warm_in",
                                      tag="warm_in")
            warm_ps = psum_pool.tile([128, 64], fp32, name="warm_ps",
                                     tag="warm_ps", bufs=1)
            nc.gpsimd.memset(warm_in[:], 0.0)
            for _ in range(62):
                nc.tensor.matmul(warm_ps[:], warm_in[:], warm_in[:, 0:64])

            # Double-buffered padded input images (pad arrives from host).
            xp = [[xp_pool.tile([128, HP, WP], fp16, name=f"xp{par}_{kt}",
                                tag=f"xp{par}_{kt}")
                   for kt in range(KT)] for par in range(2)]

            xap = x_d.ap()
            oap = o_d.ap()
            wap = w_d.ap()

            def xdma(eng, n, par, kt, r0, r1):
                eng.dma_start(xp[par][kt][:, r0:r1, :],
                              xap[n, kt * 128:(kt + 1) * 128, r0:r1, :])

            # Head: completions on a ring serialize at ~1.5-2us each, and a
            # mid-stream matmul stall also re-throttles the PE clock for
            # ~3.4us at half rate.  The four operands of the first 18 real
            # matmuls (band-0 of both kt, mt0 weights of both kt) are spread
            # over FOUR queues (sync/scalar HWDGE + gpsimd/vector SWDGE) so
            # they complete in parallel ~2us before first use.
            xdma(nc.sync, 0, 0, 0, 0, 10)                      # band0 kt0
            nc.scalar.dma_start(w_sb[:, 0, 0], wap[:, 0, 0])   # w kt0 mt0
            xdma(nc.gpsimd, 0, 0, 1, 0, 10)                    # band0 kt1
            nc.scalar.dma_start(w_sb[:, 1, 0], wap[:, 1, 0])   # w kt1 mt0
            nc.sync.dma_start(w_sb[:, 0, 1], wap[:, 0, 1])     # w kt0 mt1
            nc.scalar.dma_start(w_sb[:, 1, 1], wap[:, 1, 1])   # w kt1 mt1
            xdma(nc.sync, 0, 0, 0, 10, 26)                     # rows 10:26
            xdma(nc.scalar, 0, 0, 1, 10, 26)
            nc.sync.dma_start(b_sb[:], b_d.ap())               # bias
            xdma(nc.scalar, 0, 0, 1, 26, 58)                   # rows 26:58
            xdma(nc.sync, 0, 0, 0, 26, 58)

            def drain(n, mt, y0, ps, rows=RB):
                ot = out_pool.tile([128, rows, W], fp16, name="ot", tag="ot")
                nc.vector.tensor_scalar_add(ot[:], ps[:], b_sb[:, mt:mt + 1])
                ring(mt).dma_start(
                    oap[n, mt * 128:(mt + 1) * 128, y0:y0 + rows, :], ot[:])

            def chunk_mms(par, mt, y0, ps, rows=RB):
                first = True
                for kt in range(KT):
                    for dy in range(K1):
                        for dx in range(K2):
                            pos = dy * K2 + dx
                            last = (kt == KT - 1 and pos == POS - 1)
                            nc.tensor.matmul(
                                ps[:, :, :],
                                w_sb[:, kt, mt, pos, :],
                                xp[par][kt][:, y0 + dy:y0 + dy + rows,
                                            dx:dx + W],
                                start=first, stop=last,
                            )
                            first = False

            # Chunk 0 of image 0 opens the stream: interleave its four
            # (mt, kt) blocks kt-major across two PSUM banks so the kt=1
            # operands (which land 2nd/3rd on their queues) are not needed
            # until ~18 matmuls in (~+3.4us of slack vs kt-inner order).
            ps_c0 = [psum_pool.tile([128, RB, W], fp32, name="ps", tag="ps")
                     for _ in range(MT)]
            for kt in range(KT):
                for mt in range(MT):
                    for dy in range(K1):
                        for dx in range(K2):
                            pos = dy * K2 + dx
                            nc.tensor.matmul(
                                ps_c0[mt][:, :, :],
                                w_sb[:, kt, mt, pos, :],
                                xp[0][kt][:, dy:dy + RB, dx:dx + W],
                                start=(kt == 0 and pos == 0),
                                stop=(kt == KT - 1 and pos == POS - 1),
                            )
                    # Junk cushion between blocks: absorbs head-DMA jitter
                    # ahead of the next block's semaphore wait (a stalled PE
                    # both idles and risks re-throttling the clock gate).
                    for _ in range(5):
                        nc.tensor.matmul(warm_ps[:], warm_in[:],
                                         warm_in[:, 0:64])
            for mt in range(MT):
                drain(0, mt, 0, ps_c0[mt])

            for n in range(N_PER_CORE):
                par = n % 2
                if n > 0:
                    for kt in range(KT):
                        xdma(ring(kt), n, par, kt, 0, 58)      # whole image
                for ch in range(1 if n == 0 else 0, NCH):
                    y0 = ch * RB
                    # Final chunk of the final image: split into half-chunks
                    # so the last drain + output DMA (on the span's critical
                    # path: ~0.7us vector + ~2.6us DMA round-trip) overlaps
                    # the trailing matmuls.
                    halve = (n == N_PER_CORE - 1 and ch == NCH - 1)
                    for mt in range(MT):
                        if halve:
                            for h in range(2):
                                hr = RB // 2
                                ps = psum_pool.tile([128, hr, W], fp32,
                                                    name="ps", tag="ps")
                                chunk_mms(par, mt, y0 + h * hr, ps, rows=hr)
                                drain(n, mt, y0 + h * hr, ps, rows=hr)
                        else:
                            ps = psum_pool.tile([128, RB, W], fp32,
                                                name="ps", tag="ps")
                            chunk_mms(par, mt, y0, ps)
                            drain(n, mt, y0, ps)
    nc.compile()
    return nc


def get_nc():
    if "nc" not in _NC_CACHE:
        _NC_CACHE["nc"] = _build_nc()
    return _NC_CACHE["nc"]


def kernel(x, weight, alphas, betas, bias):
    from concourse.bass_utils import run_bass_kernel_spmd

    xpad, w_arr, b_arr = _host_prep(x, weight, alphas, betas, bias)
    nc = get_nc()
    in_maps = [
        {"x": xpad[i * N_PER_CORE:(i + 1) * N_PER_CORE], "w": w_arr,
         "b": b_arr}
        for i in range(N_CORES)
    ]
    res = run_bass_kernel_spmd(nc, in_maps, core_ids=list(range(N_CORES)))
    LAST_RESULT["res"] = res
    return np.concatenate([r["out"] for r in res.results],
                          axis=0).astype(np.float32)
